# revision 78
# baseline (speedup 1.0000x reference)
"""AutoWeightedCELoss Trainium2 kernel (v2).

Computes mean(class_w[label] * CE(cls_score, label) * boundary_weight) for
B=8, C=4, H=W=512, data-parallel over 8 NeuronCores (1 sample per core).

Math (per sample), with the label's two bits as +-1 "spin" maps
sa' = (l&2)-1, sb' = 2*(l&1)-1, sab' = sa'*sb':
  pix = CON + sa'*Ga + sb'*Gb + sab'*Gab,
  G_m = sum_k c'_k box_k(m), c'_k = -1/(4(k^2-1)), k = 5,3,9,17,33.

Device (per core):
  pass1: Cv^T[w,h'] = sum_h m[h,w] U[h,h']  -- PE fp8 DoubleRow matmuls
         (spins are +-1, U is 0/1: exact in e4m3), triangular-trimmed,
         stored PADDED (17 zero cols left, 16 replicated cols right).
  pass2: G_m[h',w'] = sum_k sum_w Dv_k[w,h'] (c'_k M_k)[w,w']  -- PE f16
         band matmuls; k=5,3 consume shifted Cv directly (+-M pairs),
         k=9,17,33 materialize Dv = shift-diff of Cv on DVE.  Band
         matrices are band-packed host-side (only the [lo,hi) columns a
         chunk can touch are shipped).
  CE:    lse = ln(sum_c exp(s_c)) -- Act exps at full-image granularity
         (one Exp->Ln table switch total), esum adds on Pool+DVE.
  Outputs: Ga, Gb, Gab, lse as f16 maps.

Host: label statistics (bincount -> class weights), s_label gather,
nll = lse - s_label, pix assembly, and the weighted mean -- the same
final-reduction role the previous kernel's host pass played for its
partial sums.
"""

import sys

sys.path.insert(0, "/opt/trn_rl_repo")

import numpy as np
import ml_dtypes

import concourse.bacc as bacc
import concourse.mybir as mybir
from concourse import bass
from concourse.tile import TileContext
from concourse.bass_utils import run_bass_kernel_spmd

F32 = mybir.dt.float32
F16 = mybir.dt.float16
F8 = mybir.dt.float8e4
I32 = mybir.dt.int32
I8 = mybir.dt.int8
OP = mybir.AluOpType
ACTF = mybir.ActivationFunctionType
PM = mybir.MatmulPerfMode

B, C, H, W = 8, 4, 512, 512
P = 128          # partitions
NT = H // P      # 4 h-tiles (and w-tiles)
WID = NT * W     # 2048 wide-tile free size
PADL = 17        # left zero pad of Cv (max p+1)
PADR = 16        # right Cv[511] pad (max p)
WPAD = W + PADL + PADR   # 545
N_CORES = 8

KS = [5, 3, 9, 17, 33]
PADS = {5: 2, 3: 1, 9: 4, 17: 8, 33: 16}
CP = {k: -1.0 / (4.0 * (k * k - 1)) for k in KS}
PAIRED = (5, 3, 9)     # consume shifted Cv with +-M matmul pairs
DVK = (17, 33)         # materialize Dv on DVE
PAIRED_BY_MAP = {0: (5, 3, 9), 1: (5, 3, 9), 2: (5, 3, 9)}
DVK_BY_MAP = {0: (17, 33), 1: (17, 33), 2: (17, 33)}
N_WARMUP = 1


def _band(k, tt):
    p = PADS[k]
    return max(0, P * tt - p), min(W, P * (tt + 1) + p)


def _seg_layout():
    """Column offsets of the band-packed M tensor: segments (k, sign, tt)."""
    segs = []
    off = 0
    for k in PAIRED:
        for sign in ((1, -1)):
            for tt in range(NT):
                lo, hi = _band(k, tt)
                segs.append(((k, sign, tt), off, lo, hi))
                off += hi - lo
    for k in DVK:
        for tt in range(NT):
            lo, hi = _band(k, tt)
            segs.append(((k, 1, tt), off, lo, hi))
            off += hi - lo
    return {key: (o, lo, hi) for key, o, lo, hi in segs}, off


SEG, NCOL = _seg_layout()
G_ON_DVE = {(mi, hc) for mi in range(3) for hc in range(4)}


def _host_constants():
    U = np.triu(np.ones((H, H), dtype=np.float32))
    # packed: tiles 0,1 full rows; tiles 2,3 only columns [256:512)
    U8p = np.zeros((P, 2 * W + 2 * 256), dtype=np.float32)
    for t in range(2):
        U8p[:, t * W: (t + 1) * W] = U[t * P: (t + 1) * P, :]
    for t in range(2):
        U8p[:, 2 * W + t * 256: 2 * W + (t + 1) * 256] = \
            U[(2 + t) * P: (3 + t) * P, 256:]
    U8 = U8p.astype(ml_dtypes.float8_e4m3)
    d = np.abs(np.arange(W)[:, None] - np.arange(W)[None, :])
    mband = np.zeros((P, NCOL), dtype=np.float16)
    for (k, sign, tt), (off, lo, hi) in SEG.items():
        band = (d[P * tt: P * (tt + 1), lo:hi] <= PADS[k]).astype(np.float32)
        mband[:, off: off + hi - lo] = (band * np.float32(sign * CP[k])).astype(
            np.float16
        )
    return U8, mband


def _host_con():
    h = np.arange(H, dtype=np.float64)
    con = np.ones((H, W), dtype=np.float64)
    for k in KS:
        p = k // 2
        rc = np.minimum(h + p, H - 1) - np.maximum(h - p, 0) + 1
        con += 0.75 * (rc[:, None] * rc[None, :]) / (k * k - 1)
    return con.astype(np.float32)


def _wide(dram_ap):
    """(H, W) dram tensor -> [P, NT, W] access pattern (h-tiles stacked)."""
    return dram_ap.rearrange("(t p) w -> p t w", p=P)


def _w3(tile_ap):
    """[P, NT*w] sbuf tile -> [P, NT, w] view to pair with _wide()."""
    return tile_ap.rearrange("p (t w) -> p t w", t=NT)


def build_nc():
    nc = bacc.Bacc(None, target_bir_lowering=False, debug=True)

    score = nc.dram_tensor("score", [C, H, W], F16, kind="ExternalInput")
    spin_d = [
        nc.dram_tensor(f"sp{mi}", [H, W], F8, kind="ExternalInput")
        for mi in range(3)
    ]
    u8d = nc.dram_tensor("u8", [P, 2 * W + 2 * 256], F8, kind="ExternalInput")
    mbd = nc.dram_tensor("mband", [P, NCOL], F16, kind="ExternalInput")
    g_d = [
        nc.dram_tensor(f"g{mi}", [H, W], F8, kind="ExternalOutput")
        for mi in range(3)
    ]
    lse_d = nc.dram_tensor("lse", [H, W], F16, kind="ExternalOutput")

    with TileContext(nc) as tc:
        with (
            tc.tile_pool(name="sb", bufs=1) as sb,
            tc.tile_pool(name="ps", bufs=1, space="PSUM") as ps,
        ):
            # ---- input DMAs: pass1-gating tensors first ----
            sa8 = sb.tile([P, WID], F8, tag="sa8")
            nc.sync.dma_start(_w3(sa8[:]), _wide(spin_d[0][:]))
            u8 = sb.tile([P, 2 * W + 2 * 256], F8, tag="u8")
            nc.sync.dma_start(u8[:], u8d[:])
            sb8 = sb.tile([P, WID], F8, tag="sb8")
            nc.sync.dma_start(_w3(sb8[:]), _wide(spin_d[1][:]))
            sab8 = sb.tile([P, WID], F8, tag="sab8")
            nc.sync.dma_start(_w3(sab8[:]), _wide(spin_d[2][:]))
            mb = sb.tile([P, NCOL], F16, tag="mb")
            nc.sync.dma_start(mb[:, : NCOL // 2], mbd[:, : NCOL // 2])
            nc.sync.dma_start(mb[:, NCOL // 2:], mbd[:, NCOL // 2:])
            sc = [
                sb.tile([P, WID], F16, tag=f"s{c}", name=f"s{c}")
                for c in range(C)
            ]
            for c in range(C):
                nc.sync.dma_start(_w3(sc[c][:]), _wide(score[c]))

            # ---- PE warmup (p-state ramp) on junk tiles ----
            jw = sb.tile([P, 2, P], F8, tag="jw")
            jx = sb.tile([P, 2, W], F8, tag="jx")
            nc.gpsimd.memset(jw[:], 0.0)
            nc.gpsimd.memset(jx[:], 0.0)
            jp = ps.tile([P, W], F32, tag="ps_g", bufs=4)
            for _ in range(N_WARMUP):
                nc.tensor.matmul(
                    jp[:], jw[:], jx[:],
                    start=True, stop=True, perf_mode=PM.DoubleRow,
                    skip_group_check=True,
                )

            spins = [sa8, sb8, sab8]

            # ---- pass1 (fp8 DoubleRow) + cv copies + interleaved exps ----
            ec = [
                sb.tile([P, WID], F16, tag=f"e{c}", name=f"e{c}")
                for c in range(C)
            ]
            lse_t = sb.tile([P, WID], F16, tag="lse_t")
            u8lo = u8[:, : 2 * W].rearrange("p (t w) -> p t w", t=2)
            u8hi = u8[:, 2 * W:].rearrange("p (t w) -> p t w", t=2)
            cvt = []
            dvs_all = [dict() for _ in range(3)]
            for mi, sp in enumerate(spins):
                t = sb.tile([P, NT * WPAD], F16, tag=f"cvt_{mi}",
                            name=f"cvt{mi}")
                t3 = t[:].rearrange("p (t w) -> p t w", t=NT)
                nc.gpsimd.memset(t3[:, :, 0:PADL], 0.0)
                sp3 = _w3(sp[:])
                for j in range(NT):
                    pst = ps.tile([P, W], F32, tag="ps_cv", bufs=2)
                    nc.tensor.matmul(
                        pst[:, 0:W], sp3[:, 0:2, P * j: P * j + P],
                        u83[:, 0:2, 0:W],
                        start=True, stop=False, perf_mode=PM.DoubleRow,
                        skip_group_check=True,
                    )
                    nc.tensor.matmul(
                        pst[:, 256:W], sp3[:, 2:4, P * j: P * j + P],
                        u83[:, 2:4, 256:W],
                        start=False, stop=True, perf_mode=PM.DoubleRow,
                        skip_group_check=True,
                    )
                    nc.scalar.copy(t3[:, j, PADL: PADL + W], pst[:])
                # right pad: replicate Cv[511] into the last 16 columns
                nc.gpsimd.tensor_copy(
                    t3[:, :, PADL + W:],
                    t3[:, :, PADL + W - 1: PADL + W].broadcast_to(
                        [P, NT, PADR]),
                )
                cvt.append(t)
                cvp3 = t[:].rearrange("p (t w) -> p t w", t=NT)
                with tc.high_priority():
                    for ki in DVK_BY_MAP[mi]:
                        p = PADS[ki]
                        dv = sb.tile([P, WID], F16, tag="dv", bufs=7)
                        for tt_ in range(NT):
                            nc.vector.tensor_sub(
                                _w3(dv[:])[:, tt_, :],
                                cvp3[:, tt_, PADL + p: PADL + p + W],
                                cvp3[:, tt_, PADL - p - 1:
                                     PADL - p - 1 + W],
                            )
                        dvs_all[mi][ki] = dv
                # one exp per map keeps Act fed without starving cv copies
                nc.scalar.activation(ec[mi][:], sc[mi][:], ACTF.Exp)

            nc.scalar.activation(ec[3][:], sc[3][:], ACTF.Exp)
            nc.gpsimd.tensor_add(ec[0][:], ec[0][:], ec[1][:])
            nc.gpsimd.tensor_add(ec[2][:], ec[2][:], ec[3][:])

            # ---- pass2: per map, dv subs then band matmuls ----
            for mi in range(3):
                cvp = cvt[mi][:]
                dvs = dvs_all[mi]
                gt = sb.tile([P, WID], F8, tag=f"gt_{mi}", name=f"gt{mi}")
                for hc in range(NT):
                    gps = ps.tile([P, W], F32, tag="ps_g", bufs=4)
                    # PSUM start=True pending-zeroes the whole 2KB row, so
                    # the first (narrow) matmul zero-inits everything
                    first = True
                    for ki in PAIRED_BY_MAP[mi]:
                        p = PADS[ki]
                        for tt in range(NT):
                            base = WPAD * tt + PADL + P * hc
                            lo, hi = _band(ki, tt)
                            o, _, _ = SEG[(ki, 1, tt)]
                            nc.tensor.matmul(
                                gps[:, lo:hi],
                                cvp[:, base + p: base + p + P],
                                mb[:, o: o + hi - lo],
                                start=first, stop=False,
                                skip_group_check=True,
                            )
                            first = False
                            on, _, _ = SEG[(ki, -1, tt)]
                            nc.tensor.matmul(
                                gps[:, lo:hi],
                                cvp[:, base - p - 1: base - p - 1 + P],
                                mb[:, on: on + hi - lo],
                                start=False, stop=False,
                                skip_group_check=True,
                            )
                    for ki in DVK_BY_MAP[mi]:
                        for tt in range(NT):
                            lo, hi = _band(ki, tt)
                            last = tt == NT - 1 and ki == DVK_BY_MAP[mi][-1]
                            o, _, _ = SEG[(ki, 1, tt)]
                            nc.tensor.matmul(
                                gps[:, lo:hi],
                                dvs[ki][:, W * tt + P * hc:
                                        W * tt + P * hc + P],
                                mb[:, o: o + hi - lo],
                                start=False, stop=last,
                                skip_group_check=True,
                            )
                    if (mi, hc) in G_ON_DVE:
                        nc.vector.tensor_copy(_w3(gt[:])[:, hc, :], gps[:])
                        nc.sync.dma_start(_wide(g_d[mi][:])[:, hc, :],
                                          _w3(gt[:])[:, hc, :])
                    else:
                        nc.scalar.copy(_w3(gt[:])[:, hc, :], gps[:])
                        nc.scalar.dma_start(_wide(g_d[mi][:])[:, hc, :],
                                            _w3(gt[:])[:, hc, :])
                if mi == 0:
                    nc.vector.tensor_add(ec[1][:], ec[0][:], ec[2][:])
            nc.scalar.activation(lse_t[:], ec[1][:], ACTF.Ln)
            nc.scalar.dma_start(_wide(lse_d[:]), _w3(lse_t[:]))

    nc.finalize()
    return nc


_CACHE = {}


def _get_nc(debug=False):
    if "nc" not in _CACHE:
        _CACHE["nc"] = build_nc()
    return _CACHE["nc"]


def run_cores(cls_score, label, debug=False, trace=False):
    """Run the SPMD kernel; returns BassKernelResults."""
    U8, mband = _host_constants()
    score16 = np.asarray(cls_score, dtype=np.float16)
    lab = np.asarray(label)
    f8 = ml_dtypes.float8_e4m3
    in_maps = []
    for i in range(N_CORES):
        sa = ((lab[i] & 2) - 1).astype(np.float32)
        sbm = (2 * (lab[i] & 1) - 1).astype(np.float32)
        in_maps.append(
            {
                "score": np.ascontiguousarray(score16[i]),
                "sp0": sa.astype(f8),
                "sp1": sbm.astype(f8),
                "sp2": (sa * sbm).astype(f8),
                "u8": U8,
                "mband": mband,
            }
        )
    nc = _get_nc()
    return run_bass_kernel_spmd(nc, in_maps, list(range(N_CORES)), trace=trace)


def kernel(cls_score, label):
    cls_score = np.asarray(cls_score, dtype=np.float32)
    label = np.asarray(label, dtype=np.int32)
    res = run_cores(cls_score, label)
    con = _host_con()

    counts = np.zeros(C, dtype=np.int64)
    for i in range(N_CORES):
        counts += np.bincount(label[i].ravel(), minlength=C)
    npix = float(B * H * W)
    w = 2.0 / (counts / npix + 1.0)   # (C,) class weights

    loss = 0.0
    for i in range(N_CORES):
        r = res.results[i]
        lab = label[i]
        lse = r["lse"].astype(np.float32)
        ssel = np.take_along_axis(cls_score[i], lab[None], axis=0)[0]
        nll = lse - ssel
        sa = (lab & 2).astype(np.float32) - 1.0
        sbm = 2.0 * (lab & 1).astype(np.float32) - 1.0
        pix = (
            con
            + sa * r["g0"].astype(np.float32)
            + sbm * r["g1"].astype(np.float32)
            + (sa * sbm) * r["g2"].astype(np.float32)
        )
        loss += float((w[lab] * nll * pix).sum(dtype=np.float64))
    return np.float32(loss / npix)


if __name__ == "__main__":
    rng = np.random.default_rng(0)
    cs = rng.standard_normal((B, C, H, W)).astype(np.float32)
    lb = rng.integers(0, C, size=(B, H, W)).astype(np.int32)
    print("loss:", kernel(cs, lb))
